# revision 4
# baseline (speedup 1.0000x reference)
"""Trainium2 Bass kernel for nn_Dist2CycleLayer.

Computes out = relu(adjacency * Linv) @ W.T + b  with N = 8192.
(x_e is an input of the nn.Module but is discarded by its forward pass,
so it is never shipped to the device.)

Sharding: row-partition the [N, N] matrices across 8 NeuronCores
(1024 output rows per core); the column reduction is row-local.

Layout + quantization (the rel-err budget is 2e-2; measured ~1e-2):
  - adjacency in [0,1) is quantized to uint8 (a ~= qa/255).
  - Linv ~ N(0,1) is quantized to int8 with scale DELTA=4/127.
  - Both are stored TRANSPOSED per core: the contraction axis j (columns
    of the original matrices) lands on SBUF partitions, so that
      * relu is a tensor_scalar (DVE 4x mode),
      * the Hadamard product is a tensor_tensor (DVE 2x mode),
      * the dot with W becomes a PE matmul with stationary w2[128,1]
        per j-chunk, accumulating [1,512] fp32 PSUM tiles over all 64
        chunks (partition-axis reduction for free on TensorE).
  - Host pre-arranges each core's transposed slice as [16, 128, 4096]
    (group g, partition p, free = (c4, i)), j = g*512 + c4*128 + p, so
    every SBUF tile is one contiguous 512KB HBM read.

Engine roles per [128,4096] tile (16 tiles/core/rep):
  SP  HWDGE : qa tile u8 DMA (512KB)
  POOL SWDGE: ql tile s8 -> f16 casting DMA (512KB read, 1MB write)
  ACT       : a16 = float(qa)  (u8->f16 copy, 1x dtype-independent)
  DVE       : t = max(l16, 0) (4x); m = a16 * t (2x)
  PE        : psum[1,512] += w2[:,cc].T @ m[:,512-slice] (x8)
Final: out = psum * (1/255^2) + b  (w2 = W*DELTA*255 keeps f16 normal),
one [1,1024] f32 DMA per core per rep.

Roofline: 16.8MB HBM reads/core (47us @358GB/s), 25.2MB SBUF writes
(58us @435GB/s), ACT 59us, DVE 53us, PE 28us -> ~60us target vs 178us
fp32 baseline.
"""

import os

import numpy as np

N = 8192
N_CORES = 8
ROWS = N // N_CORES  # 1024 output rows per core
P = 128
G = 4  # j-chunks per tile group
NG = N // (P * G)  # 16 tile groups per core
FREE = G * ROWS  # 4096 free elements per tile
NCHUNK = N // P  # 64 j-chunks
DELTA = 4.0 / 127.0
WSCALE = 255.0  # keeps w2 = W*DELTA*WSCALE out of f16 subnormals
OUT_SHAPE = (1, ROWS)

IO_BUFS = int(os.environ.get("K_IO_BUFS", "3"))
# K_LINV_F16=1: ship Linv as f16 from host (no SWDGE cast DMA) — A/B
# fallback in case the casting DMA is slow or wrong.
LINV_F16 = os.environ.get("K_LINV_F16", "0") == "1"
# Number of the 16 tile groups whose relu runs on ACT instead of DVE
# (DVE TS relu is ~1.13us/tile; ACT has slack next to the adj dequant).
RELU_ACT = int(os.environ.get("K_RELU_ACT", "0"))
_RELU_ACT_SET = {round(i * 16 / RELU_ACT) for i in range(RELU_ACT)} if RELU_ACT else set()

_CACHE = {}


def _build(reps=1):
    import concourse.bacc as bacc
    import concourse.mybir as mybir
    from concourse import tile
    from concourse.bass import MemorySpace

    f32 = mybir.dt.float32
    f16 = mybir.dt.float16
    u8 = mybir.dt.uint8
    s8 = mybir.dt.int8
    Alu = mybir.AluOpType

    nc = bacc.Bacc(
        "TRN2",
        target_bir_lowering=False,
        debug=False,
        num_devices=N_CORES,
    )

    adjq = nc.dram_tensor("adjq", [NG, P, FREE], u8, kind="ExternalInput").ap()
    linvq = nc.dram_tensor(
        "linvq", [NG, P, FREE], f16 if LINV_F16 else s8, kind="ExternalInput"
    ).ap()
    w2 = nc.dram_tensor("w2", [P, NCHUNK], f16, kind="ExternalInput").ap()
    b = nc.dram_tensor("b", [1, 1], f32, kind="ExternalInput").ap()
    out = nc.dram_tensor("out", [1, ROWS], f32, kind="ExternalOutput").ap()

    with tile.TileContext(nc) as tc:
        with (
            tc.tile_pool(name="consts", bufs=1) as consts,
            tc.tile_pool(name="io", bufs=IO_BUFS) as io,
            tc.tile_pool(name="work", bufs=IO_BUFS) as work,
            tc.tile_pool(name="psum", bufs=2, space=MemorySpace.PSUM) as psum,
            tc.tile_pool(name="small", bufs=2) as small,
        ):
            w2t = consts.tile([P, NCHUNK], f16)
            nc.sync.dma_start(out=w2t[:], in_=w2)
            bt = consts.tile([1, 1], f32)
            nc.sync.dma_start(out=bt[:], in_=b)

            for rep in range(reps):
                ps_a = psum.tile([1, 512], f32, tag="ps_a")
                ps_b = psum.tile([1, 512], f32, tag="ps_b")
                for g in range(NG):
                    qa_t = io.tile([P, FREE], u8, tag="qa")
                    nc.sync.dma_start(out=qa_t[:], in_=adjq[g])
                    if LINV_F16:
                        l16_t = io.tile([P, FREE], f16, tag="l16")
                        nc.scalar.dma_start(out=l16_t[:], in_=linvq[g])
                    else:
                        l16_t = io.tile([P, FREE], f16, tag="l16")
                        nc.gpsimd.dma_start(out=l16_t[:], in_=linvq[g])
                    a16_t = work.tile([P, FREE], f16, tag="a16")
                    nc.scalar.copy(out=a16_t[:], in_=qa_t[:])
                    t_t = work.tile([P, FREE], f16, tag="t")
                    if g in _RELU_ACT_SET:
                        nc.scalar.activation(
                            t_t[:], l16_t[:], mybir.ActivationFunctionType.Relu
                        )
                    else:
                        nc.vector.tensor_scalar_max(t_t[:], l16_t[:], 0.0)
                    m_t = work.tile([P, FREE], f16, tag="m")
                    nc.vector.tensor_mul(out=m_t[:], in0=a16_t[:], in1=t_t[:])
                    for c4 in range(G):
                        cc = g * G + c4
                        first = cc == 0
                        last = cc == NCHUNK - 1
                        base = c4 * ROWS
                        nc.tensor.matmul(
                            ps_a[:],
                            w2t[:, cc : cc + 1],
                            m_t[:, base : base + 512],
                            start=first,
                            stop=last,
                        )
                        nc.tensor.matmul(
                            ps_b[:],
                            w2t[:, cc : cc + 1],
                            m_t[:, base + 512 : base + 1024],
                            start=first,
                            stop=last,
                        )
                stage = small.tile([1, ROWS], f32, tag="stage")
                inv = 1.0 / (255.0 * WSCALE)
                nc.vector.tensor_scalar(
                    out=stage[:, 0:512],
                    in0=ps_a[:],
                    scalar1=inv,
                    scalar2=bt[:],
                    op0=Alu.mult,
                    op1=Alu.add,
                )
                nc.vector.tensor_scalar(
                    out=stage[:, 512:1024],
                    in0=ps_b[:],
                    scalar1=inv,
                    scalar2=bt[:],
                    op0=Alu.mult,
                    op1=Alu.add,
                )
                nc.sync.dma_start(out=out, in_=stage[:])

    nc.compile()
    return nc


def get_nc(reps=1):
    key = ("nc", reps)
    if key not in _CACHE:
        _CACHE[key] = _build(reps)
    return _CACHE[key]


def _tileize(mat_core):
    """[N, ROWS] transposed core slice -> [NG, P, FREE] tile layout.

    j = g*512 + c4*128 + p  ->  [g, p, (c4, i)]
    """
    x = mat_core.reshape(NG, G, P, ROWS)  # [g, c4, p, i]
    x = x.transpose(0, 2, 1, 3)  # [g, p, c4, i]
    return np.ascontiguousarray(x.reshape(NG, P, FREE))


def make_in_maps(adjacency, Linv, W, b):
    adjacency = np.asarray(adjacency, dtype=np.float32)
    Linv = np.asarray(Linv, dtype=np.float32)
    W = np.asarray(W, dtype=np.float32).reshape(1, N)
    b = np.asarray(b, dtype=np.float32).reshape(1, 1)

    qa = np.rint(adjacency * 255.0).astype(np.uint8)  # a ~= qa/255
    ql = np.clip(np.rint(Linv * (1.0 / DELTA)), -127, 127).astype(np.int8)

    w2 = (W.reshape(NCHUNK, P).T * (DELTA * WSCALE)).astype(np.float16)
    w2 = np.ascontiguousarray(w2)  # [P, NCHUNK]

    in_maps = []
    for c in range(N_CORES):
        r0, r1 = c * ROWS, (c + 1) * ROWS
        at = np.ascontiguousarray(qa[r0:r1, :].T)  # [N, ROWS] u8
        lt = np.ascontiguousarray(ql[r0:r1, :].T)  # [N, ROWS] s8
        linv_tiles = _tileize(lt)
        if LINV_F16:
            linv_tiles = linv_tiles.astype(np.float16)
        in_maps.append(
            {
                "adjq": _tileize(at),
                "linvq": linv_tiles,
                "w2": w2,
                "b": b,
            }
        )
    return in_maps


def unstage(core_out):
    """[1, ROWS] device output -> [ROWS, 1] output rows for one core."""
    return np.asarray(core_out, dtype=np.float32).reshape(ROWS, 1)


def kernel(x_e=None, Linv=None, adjacency=None, W=None, b=None, **_unused):
    from concourse.bass_utils import run_bass_kernel_spmd

    nc = get_nc()
    in_maps = make_in_maps(adjacency, Linv, W, b)
    res = run_bass_kernel_spmd(nc, in_maps, core_ids=list(range(N_CORES)))
    out = np.concatenate([unstage(r["out"]) for r in res.results], axis=0)
    return out.astype(np.float32)


# revision 6
# speedup vs baseline: 1.0628x; 1.0628x over previous
"""Trainium2 Bass kernel for nn_Dist2CycleLayer.

Computes out = relu(adjacency * Linv) @ W.T + b  with N = 8192.
(x_e is an input of the nn.Module but is discarded by its forward pass,
so it is never shipped to the device.)

Sharding: row-partition the [N, N] matrices across 8 NeuronCores
(1024 output rows per core); the column reduction is row-local.

Layout + quantization (the rel-err budget is 2e-2; measured ~1e-2):
  - adjacency in [0,1) is quantized to uint8 (a ~= qa/255).
  - Linv ~ N(0,1) is quantized to int8 with scale DELTA=4/127.
  - Both are stored TRANSPOSED per core: the contraction axis j (columns
    of the original matrices) lands on SBUF partitions, so that
      * relu is a tensor_scalar (DVE 4x mode),
      * the Hadamard product is a tensor_tensor (DVE 2x mode),
      * the dot with W becomes a PE matmul with stationary w2[128,1]
        per j-chunk, accumulating [1,512] fp32 PSUM tiles over all 64
        chunks (partition-axis reduction for free on TensorE).
  - Host pre-arranges each core's transposed slice as [16, 128, 4096]
    (group g, partition p, free = (c4, i)), j = g*512 + c4*128 + p, so
    every SBUF tile is one contiguous 512KB HBM read.

Engine roles per [128,4096] tile (16 tiles/core/rep):
  SP  HWDGE : qa tile u8 DMA (512KB)
  POOL SWDGE: ql tile s8 -> f16 casting DMA (512KB read, 1MB write)
  ACT       : a16 = float(qa)  (u8->f16 copy, 1x dtype-independent)
  DVE       : t = max(l16, 0) (4x); m = a16 * t (2x)
  PE        : psum[1,512] += w2[:,cc].T @ m[:,512-slice] (x8)
Final: out = psum * (1/255^2) + b  (w2 = W*DELTA*255 keeps f16 normal),
one [1,1024] f32 DMA per core per rep.

Roofline: 16.8MB HBM reads/core (47us @358GB/s), 25.2MB SBUF writes
(58us @435GB/s), ACT 59us, DVE 53us, PE 28us -> ~60us target vs 178us
fp32 baseline.
"""

import os

import numpy as np

N = 8192
N_CORES = 8
ROWS = N // N_CORES  # 1024 output rows per core
P = 128
G = 4  # j-chunks per tile group
NG = N // (P * G)  # 16 tile groups per core
FREE = G * ROWS  # 4096 free elements per tile
NCHUNK = N // P  # 64 j-chunks
DELTA = 4.0 / 127.0
WSCALE = 255.0  # keeps w2 = W*DELTA*WSCALE out of f16 subnormals
OUT_SHAPE = (1, ROWS)

IO_BUFS = int(os.environ.get("K_IO_BUFS", "3"))
# K_LINV_F16=1: ship Linv as f16 from host (no SWDGE cast DMA) — A/B
# fallback in case the casting DMA is slow or wrong.
LINV_F16 = os.environ.get("K_LINV_F16", "0") == "1"
# Number of the 16 tile groups whose relu runs on ACT instead of DVE
# (DVE TS relu is ~1.13us/tile; ACT relu is ~2.9us/tile marginal).
RELU_ACT = int(os.environ.get("K_RELU_ACT", "0"))
_RELU_ACT_SET = {round(i * 16 / RELU_ACT) for i in range(RELU_ACT)} if RELU_ACT else set()
# Number of the 16 adj tile groups loaded via SWDGE casting DMA
# (u8 HBM -> f16 SBUF, no ACT pass needed for those tiles).
ADJ_CAST = int(os.environ.get("K_ADJ_CAST", "0"))
_ADJ_CAST_SET = (
    {round(i * 16 / ADJ_CAST + 0.49) % 16 for i in range(ADJ_CAST)}
    if ADJ_CAST
    else set()
)

_CACHE = {}


def _build(reps=1):
    import concourse.bacc as bacc
    import concourse.mybir as mybir
    from concourse import tile
    from concourse.bass import MemorySpace

    f32 = mybir.dt.float32
    f16 = mybir.dt.float16
    u8 = mybir.dt.uint8
    s8 = mybir.dt.int8
    Alu = mybir.AluOpType

    nc = bacc.Bacc(
        "TRN2",
        target_bir_lowering=False,
        debug=False,
        num_devices=N_CORES,
    )

    adjq = nc.dram_tensor("adjq", [NG, P, FREE], u8, kind="ExternalInput").ap()
    linvq = nc.dram_tensor(
        "linvq", [NG, P, FREE], f16 if LINV_F16 else s8, kind="ExternalInput"
    ).ap()
    w2 = nc.dram_tensor("w2", [P, NCHUNK], f16, kind="ExternalInput").ap()
    b = nc.dram_tensor("b", [1, 1], f32, kind="ExternalInput").ap()
    out = nc.dram_tensor("out", [1, ROWS], f32, kind="ExternalOutput").ap()

    with tile.TileContext(nc) as tc:
        with (
            tc.tile_pool(name="consts", bufs=1) as consts,
            tc.tile_pool(name="io", bufs=IO_BUFS) as io,
            tc.tile_pool(name="work", bufs=IO_BUFS) as work,
            tc.tile_pool(name="psum", bufs=2, space=MemorySpace.PSUM) as psum,
            tc.tile_pool(name="small", bufs=2) as small,
        ):
            w2t = consts.tile([P, NCHUNK], f16)
            nc.sync.dma_start(out=w2t[:], in_=w2)
            bt = consts.tile([1, 1], f32)
            nc.sync.dma_start(out=bt[:], in_=b)

            for rep in range(reps):
                ps_a = psum.tile([1, 512], f32, tag="ps_a")
                ps_b = psum.tile([1, 512], f32, tag="ps_b")
                for g in range(NG):
                    l16_t = io.tile([P, FREE], f16, tag="l16")
                    if LINV_F16:
                        nc.scalar.dma_start(out=l16_t[:], in_=linvq[g])
                    else:
                        nc.gpsimd.dma_start(out=l16_t[:], in_=linvq[g])
                    a16_t = work.tile([P, FREE], f16, tag="a16")
                    if g in _ADJ_CAST_SET:
                        # u8 -> f16 cast happens inside the SWDGE DMA.
                        nc.gpsimd.dma_start(out=a16_t[:], in_=adjq[g])
                    else:
                        qa_t = io.tile([P, FREE], u8, tag="qa")
                        nc.sync.dma_start(out=qa_t[:], in_=adjq[g])
                        nc.scalar.copy(out=a16_t[:], in_=qa_t[:])
                    t_t = work.tile([P, FREE], f16, tag="t")
                    if g in _RELU_ACT_SET:
                        nc.scalar.activation(
                            t_t[:], l16_t[:], mybir.ActivationFunctionType.Relu
                        )
                    else:
                        nc.vector.tensor_scalar_max(t_t[:], l16_t[:], 0.0)
                    m_t = work.tile([P, FREE], f16, tag="m")
                    nc.vector.tensor_mul(out=m_t[:], in0=a16_t[:], in1=t_t[:])
                    for c4 in range(G):
                        cc = g * G + c4
                        first = cc == 0
                        last = cc == NCHUNK - 1
                        base = c4 * ROWS
                        nc.tensor.matmul(
                            ps_a[:],
                            w2t[:, cc : cc + 1],
                            m_t[:, base : base + 512],
                            start=first,
                            stop=last,
                        )
                        nc.tensor.matmul(
                            ps_b[:],
                            w2t[:, cc : cc + 1],
                            m_t[:, base + 512 : base + 1024],
                            start=first,
                            stop=last,
                        )
                stage = small.tile([1, ROWS], f32, tag="stage")
                inv = 1.0 / (255.0 * WSCALE)
                nc.vector.tensor_scalar(
                    out=stage[:, 0:512],
                    in0=ps_a[:],
                    scalar1=inv,
                    scalar2=bt[:],
                    op0=Alu.mult,
                    op1=Alu.add,
                )
                nc.vector.tensor_scalar(
                    out=stage[:, 512:1024],
                    in0=ps_b[:],
                    scalar1=inv,
                    scalar2=bt[:],
                    op0=Alu.mult,
                    op1=Alu.add,
                )
                nc.sync.dma_start(out=out, in_=stage[:])

    nc.compile()
    return nc


def get_nc(reps=1):
    key = ("nc", reps)
    if key not in _CACHE:
        _CACHE[key] = _build(reps)
    return _CACHE[key]


def _tileize(mat_core):
    """[N, ROWS] transposed core slice -> [NG, P, FREE] tile layout.

    j = g*512 + c4*128 + p  ->  [g, p, (c4, i)]
    """
    x = mat_core.reshape(NG, G, P, ROWS)  # [g, c4, p, i]
    x = x.transpose(0, 2, 1, 3)  # [g, p, c4, i]
    return np.ascontiguousarray(x.reshape(NG, P, FREE))


def make_in_maps(adjacency, Linv, W, b):
    adjacency = np.asarray(adjacency, dtype=np.float32)
    Linv = np.asarray(Linv, dtype=np.float32)
    W = np.asarray(W, dtype=np.float32).reshape(1, N)
    b = np.asarray(b, dtype=np.float32).reshape(1, 1)

    qa = np.rint(adjacency * 255.0).astype(np.uint8)  # a ~= qa/255
    ql = np.clip(np.rint(Linv * (1.0 / DELTA)), -127, 127).astype(np.int8)

    w2 = (W.reshape(NCHUNK, P).T * (DELTA * WSCALE)).astype(np.float16)
    w2 = np.ascontiguousarray(w2)  # [P, NCHUNK]

    in_maps = []
    for c in range(N_CORES):
        r0, r1 = c * ROWS, (c + 1) * ROWS
        at = np.ascontiguousarray(qa[r0:r1, :].T)  # [N, ROWS] u8
        lt = np.ascontiguousarray(ql[r0:r1, :].T)  # [N, ROWS] s8
        linv_tiles = _tileize(lt)
        if LINV_F16:
            linv_tiles = linv_tiles.astype(np.float16)
        in_maps.append(
            {
                "adjq": _tileize(at),
                "linvq": linv_tiles,
                "w2": w2,
                "b": b,
            }
        )
    return in_maps


def unstage(core_out):
    """[1, ROWS] device output -> [ROWS, 1] output rows for one core."""
    return np.asarray(core_out, dtype=np.float32).reshape(ROWS, 1)


def kernel(x_e=None, Linv=None, adjacency=None, W=None, b=None, **_unused):
    from concourse.bass_utils import run_bass_kernel_spmd

    nc = get_nc()
    in_maps = make_in_maps(adjacency, Linv, W, b)
    res = run_bass_kernel_spmd(nc, in_maps, core_ids=list(range(N_CORES)))
    out = np.concatenate([unstage(r["out"]) for r in res.results], axis=0)
    return out.astype(np.float32)
